# revision 5
# baseline (speedup 1.0000x reference)
"""AttentionNet kernel for 8 TRN2 NeuronCores (Bass/Tile).

Computes, for att_vectors [131072, 512], ref_vector [1,512], Wh/Wv [512,512],
Ws [1,512]:
    h = tanh(att @ Wh.T + ref @ Wv.T)
    w = softmax((h @ Ws.T)[:, 0])
    out = w @ att                                  -> [512] float32

Strategy: shard att_vectors row-wise across 8 cores (16384 rows each); each
core runs a fused single-pass pipeline over 512-row tiles:
  - att tile loaded natural layout; transposed on the PE (fp32r transpose-mode,
    16x [128,128] per tile) into PSUM, copied to SBUF by DVE.
  - pre^T[d',s] = Wh^T.T @ att^T on the PE in fp32r (1 cycle/row, ~11-bit
    mantissa at full bf16-speed; plain fp32 would be 4x slower).
  - tanh(pre + b) on ACT with per-partition bias b = ref @ Wv.T (host-computed).
  - scores = Ws . tanhT on the PE (M=2-padded stationary; M=1 is ISA-illegal
    for 4-byte dtypes); exp + per-tile Z via ACT accum_out.
  - e row->column via K=1 matmuls (out[p] = e_row[0,p] * 1), then the
    weighted sum sum_s e_s att[s,:] runs on the PE with e as the stationary
    and the natural-layout att tile as the moving operand, PSUM-accumulated
    across all 32 tiles.
Softmax normalization is deferred to the host: out = sum_c wsum_c / sum_c Z_c.
exp() needs no max-subtraction: tanh in [-1,1] and ||Ws||_2 ~ 1 bound |score|.
"""
import sys
from pathlib import Path

for _p in ("/opt/trn_rl_repo", "/root/.axon_site/_ro/trn_rl_repo"):
    if _p not in sys.path and Path(_p).is_dir():
        sys.path.insert(0, _p)

import numpy as np
import concourse.bass as bass
import concourse.mybir as mybir
from concourse import bacc
from concourse.tile import TileContext
from concourse.bass_utils import run_bass_kernel_spmd

P = 128
D = 512
KC = 4            # contraction (d) chunks of 128
MC = 4            # output (d') chunks of 128
TS = 512          # s rows per tile
S = 131072
N_CORES = 8
S_SHARD = S // N_CORES
f32 = mybir.dt.float32
f32r = mybir.dt.float32r
AF = mybir.ActivationFunctionType

_cache = {}


def _build(s_shard):
    nt = s_shard // TS
    nc = bacc.Bacc("TRN2", target_bir_lowering=False, debug=False, num_devices=1)

    att_d = nc.dram_tensor("att", [s_shard, D], f32r, kind="ExternalInput").ap()
    whT_d = nc.dram_tensor("whT", [D, D], f32r, kind="ExternalInput").ap()
    wsT_d = nc.dram_tensor("wsT", [P, MC, 2], f32r, kind="ExternalInput").ap()
    bias_d = nc.dram_tensor("bias", [P, MC], f32, kind="ExternalInput").ap()
    ident_d = nc.dram_tensor("ident", [P, P], f32r, kind="ExternalInput").ap()
    ones_d = nc.dram_tensor("ones8", [1, 8], f32r, kind="ExternalInput").ap()
    wsum_o = nc.dram_tensor("wsum_out", [2, D], f32, kind="ExternalOutput").ap()
    z_o = nc.dram_tensor("zparts", [1, nt], f32, kind="ExternalOutput").ap()

    with TileContext(nc) as tc:
        with tc.tile_pool(name="const", bufs=1) as const, \
             tc.tile_pool(name="data", bufs=3) as data, \
             tc.tile_pool(name="small", bufs=3) as small, \
             tc.tile_pool(name="stage", bufs=1) as stage, \
             tc.tile_pool(name="pre_ps", bufs=2, space="PSUM") as pre_ps, \
             tc.tile_pool(name="tr_ps", bufs=3, space="PSUM") as tr_ps, \
             tc.tile_pool(name="sc_ps", bufs=2, space="PSUM") as sc_ps, \
             tc.tile_pool(name="w_ps", bufs=1, space="PSUM") as w_ps:

            whT_sb = const.tile([P, KC, D], f32r)
            nc.sync.dma_start(whT_sb[:], whT_d.rearrange("(k p) n -> p k n", p=P))
            wsT_sb = const.tile([P, MC, 2], f32r)
            nc.sync.dma_start(wsT_sb[:], wsT_d)
            bias_sb = const.tile([P, MC], f32)
            nc.sync.dma_start(bias_sb[:], bias_d)
            ident_sb = const.tile([P, P], f32r)
            nc.sync.dma_start(ident_sb[:], ident_d)
            ones_sb = const.tile([1, 8], f32r)
            nc.sync.dma_start(ones_sb[:], ones_d)
            zparts_sb = stage.tile([1, nt], f32)

            psum_w = w_ps.tile([2, D], f32)

            for t in range(nt):
                att_sb = data.tile([P, KC, D], f32r, tag="att")
                src3 = att_d[t * TS:(t + 1) * TS, :].rearrange(
                    "(k p) d -> p k d", p=P)
                for dsp in range(2):
                    nc.sync.dma_start(
                        att_sb[:, dsp * 2:(dsp + 1) * 2, :],
                        src3[:, dsp * 2:(dsp + 1) * 2, :])

                attT_sb = data.tile([P, KC, D], f32r, tag="attT")
                for kd in range(KC):
                    ps_tr = tr_ps.tile([P, TS], f32r, tag="tr")
                    for ko in range(KC):
                        nc.tensor.transpose(
                            ps_tr[:, ko * P:(ko + 1) * P],
                            att_sb[:, ko, kd * P:(kd + 1) * P],
                            ident_sb[:])
                    nc.vector.tensor_copy(attT_sb[:, kd, :], ps_tr[:])

                tanhT_sb = data.tile([P, MC, D], f32r, tag="tanhT")
                for m in range(MC):
                    ps_pre = pre_ps.tile([P, TS], f32, tag="pre")
                    for k in range(KC):
                        nc.tensor.matmul(
                            ps_pre[:],
                            whT_sb[:, k, m * P:(m + 1) * P],
                            attT_sb[:, k, :],
                            start=(k == 0), stop=(k == KC - 1))
                    nc.scalar.activation(
                        tanhT_sb[:, m, :], ps_pre[:], AF.Tanh,
                        bias=bias_sb[:, m:m + 1], scale=1.0)

                ps_sc = sc_ps.tile([2, TS], f32, tag="sc")
                for m in range(MC):
                    nc.tensor.matmul(
                        ps_sc[:], wsT_sb[:, m, :], tanhT_sb[:, m, :],
                        start=(m == 0), stop=(m == MC - 1))

                e_row = small.tile([1, TS], f32r, tag="erow")
                nc.scalar.activation(
                    e_row[:], ps_sc[0:1, :], AF.Exp,
                    accum_out=zparts_sb[0:1, t:t + 1])

                ps_ec = sc_ps.tile([P, KC, 8], f32, tag="sc")
                for sc in range(KC):
                    nc.tensor.matmul(
                        ps_ec[:, sc, :],
                        e_row[0:1, sc * P:(sc + 1) * P],
                        ones_sb[:],
                        start=True, stop=True)
                e_col = small.tile([P, KC, 2], f32r, tag="ecol")
                nc.vector.tensor_copy(e_col[:], ps_ec[:, :, 0:2])

                for sc in range(KC):
                    nc.tensor.matmul(
                        psum_w[:], e_col[:, sc, :], att_sb[:, sc, :],
                        start=(t == 0 and sc == 0),
                        stop=(t == nt - 1 and sc == KC - 1))

            out_sb = stage.tile([2, D], f32)
            nc.vector.tensor_copy(out_sb[:], psum_w[:])
            nc.sync.dma_start(wsum_o, out_sb[:])
            nc.sync.dma_start(z_o, zparts_sb[:])
    nc.finalize()
    return nc


def _get_nc():
    if "nc" not in _cache:
        _cache["nc"] = _build(S_SHARD)
    return _cache["nc"]


def _in_maps(att_vectors, ref_vector, Wh, Wv, Ws):
    att = np.ascontiguousarray(np.asarray(att_vectors, dtype=np.float32))
    Wh = np.asarray(Wh, np.float32)
    Wv = np.asarray(Wv, np.float32)
    Ws = np.asarray(Ws, np.float32)
    ref = np.asarray(ref_vector, np.float32)

    whT = np.ascontiguousarray(Wh.T)
    b = (ref.astype(np.float64) @ Wv.T.astype(np.float64)).astype(np.float32)
    b = b.reshape(D)
    wsT = np.zeros((P, MC, 2), np.float32)
    wsT[:, :, 0] = Ws.reshape(MC, P).T
    bias = np.ascontiguousarray(b.reshape(MC, P).T)
    ident = np.eye(P, dtype=np.float32)
    ones8 = np.ones((1, 8), np.float32)

    maps = []
    for c in range(N_CORES):
        maps.append({
            "att": att[c * S_SHARD:(c + 1) * S_SHARD],
            "whT": whT,
            "wsT": wsT,
            "bias": bias,
            "ident": ident,
            "ones8": ones8,
        })
    return maps


def _combine(results):
    num = np.zeros(D, np.float64)
    den = 0.0
    for r in results:
        num += r["wsum_out"][0].astype(np.float64)
        den += float(r["zparts"].astype(np.float64).sum())
    return (num / den).astype(np.float32)


def run(trace=False, **inputs):
    """Run on hardware; returns (output, BassKernelResults)."""
    nc = _get_nc()
    maps = _in_maps(**inputs)
    res = run_bass_kernel_spmd(nc, maps, core_ids=list(range(N_CORES)), trace=trace)
    return _combine(res.results), res


def kernel(**inputs) -> np.ndarray:
    out, _ = run(**inputs)
    return out
